# revision 8
# baseline (speedup 1.0000x reference)
"""CombinedMarginLoss (ArcFace, m1=1, m2=0.5, m3=0, easy_margin) on 8 trn2 cores.

Math: loss = mean_b [ logsumexp_c(margin_logits[b,c]) - S*theta_b ] where
margin_logits[b,c] = S*logits[b,c] except the label column which is S*theta_b.
Logits are cosines in [-1, 1], so exp(S*x - S) in [e^-128, 1] needs no max pass.

The class dim is sharded across the 8 cores (partial-FC style); each core
computes per-row partial sums of exp(S*x - S) and the host does the O(B)
label/margin/log epilogue.

Device design (per core), chosen from trace analysis of the v0 kernel
(ScalarE+DVE saturated by exp+reduce work, TensorE idle, 8.8us sem-drain tail):

- Transposed layout: classes on SBUF partitions, batch rows on the free dim.
  Host ships int8-quantized logits (x8 = rint(127*x)) as a packed blob so
  every DMA is one fully contiguous [128, W] read.
- exp is computed elementwise only (no reduction work on ScalarE/DVE):
    A path (ScalarE): act Exp(S/Q * x8 - S) -> fp8 e5m2 values (sub-subnormal
      flushes to +0, which implements the tail clamp for free).
    D path (VectorE): Schraudolph bit trick in fp8: byte = rint(A'*x8 + B')
      bitcast e5m2 ~= 2^15 * exp(S*x8/Q - S).  tensor_scalar int8->int8 runs
      in the DVE 2x_2P mode (2 elem/cycle/lane).  Host pre-clamps D-chunk
      inputs at x8=86 so the byte is always in [0, 120] (no NaN patterns).
- All reductions run on the otherwise-idle TensorE: fp8 DoubleRow matmul
  against an all-ones [128,2,128] stationary contracts 256 classes/pass at
  2 fp8/cell/cycle into PSUM (fp32), accumulated across all pairs.
  Separate PSUM banks for the A and D paths (different calibration).
- Host epilogue: sums = KA*A + KD*D per row per core; KA/KD are
  HW-calibrated constants absorbing fp8 rounding/Schraudolph/act-spline bias.
"""

import numpy as np

_S = 64.0
_M2 = 0.5
_EPS = 1e-7
_NCORES = 8
_P = 128
_B = 512  # batch rows
_CS = 12500  # classes per core
_NPAIRS = 49  # ceil(12500 / 256); last pair has 44 pad classes
_CPAD = _NPAIRS * 256  # 12544

_Q = 127.0
# D path (DVE bit trick): byte = rint(SCH_A * x8 + SCH_B) bitcast e5m2.
# byte(127) = 120 (= 2^15), byte(86) = 1; host clamps D inputs at 86.
_SCH_A = float(np.float32(4 * _S * np.log2(np.e) / _Q))  # 2.9081097
_SCH_B = -249.32993
_DCLAMP = 86

# Calibration constants: sums_true ~= KA * A_dev + KD * D_dev.
# Seeded from numpy simulation; refined on HW (see calib in transcript).
_KA = 0.9903318
_KD = 0.9855134 / 32768.0

# Per-core chunk plans.  D (VectorE bit trick) takes pairs 0..29, issued on
# the Sync DMA queue; A (ScalarE act) takes pairs 30..48 on the GpSimd SWDGE
# queue.  Two queues -> the SDMA engines round-robin between the streams so
# both engines receive data continuously (a single queue serializes A behind
# D and leaves ScalarE ~8us behind at stream end).  Chunks taper small at the
# head (fast pipeline fill) and tail (short last-chunk drain).
_D_CHUNKS = [1, 3, 5, 6, 6, 5, 3, 1]   # 30 pairs
_A_CHUNKS = [1, 3, 5, 5, 3, 2]         # 19 pairs
assert sum(_D_CHUNKS) == 30 and sum(_A_CHUNKS) == 19
# Emission order (chunk index within each path), sorted by expected arrival.
_EMIT = [("D", 0), ("A", 0), ("D", 1), ("A", 1), ("D", 2), ("A", 2),
         ("D", 3), ("A", 3), ("A", 4), ("D", 4), ("A", 5), ("D", 5),
         ("D", 6), ("D", 7)]
assert sorted(i for e, i in _EMIT if e == "D") == list(range(len(_D_CHUNKS)))
assert sorted(i for e, i in _EMIT if e == "A") == list(range(len(_A_CHUNKS)))

_PAIR_ENG = ["D"] * 30 + ["A"] * 19

_nc_cache = {}


def _build_nc():
    import concourse.bacc as bacc
    import concourse.mybir as mybir
    from concourse.tile import TileContext

    nc = bacc.Bacc("TRN2", target_bir_lowering=False)
    x = nc.dram_tensor("x", [_P * _NPAIRS * 2 * _B], mybir.dt.int8,
                       kind="ExternalInput")
    out = nc.dram_tensor("sums", [2 * _B], mybir.dt.float32,
                         kind="ExternalOutput")

    # byte offset of each chunk in the blob: D chunks first, then A chunks
    d_off, a_off = [], []
    off = 0
    for n in _D_CHUNKS:
        d_off.append(off)
        off += _P * n * 2 * _B
    for n in _A_CHUNKS:
        a_off.append(off)
        off += _P * n * 2 * _B

    with TileContext(nc) as tc:
        with (
            tc.tile_pool(name="const", bufs=1) as const,
            tc.tile_pool(name="inA", bufs=len(_A_CHUNKS)) as inpA,
            tc.tile_pool(name="inD", bufs=len(_D_CHUNKS)) as inpD,
            tc.tile_pool(name="vA", bufs=len(_A_CHUNKS)) as valA,
            tc.tile_pool(name="vD", bufs=len(_D_CHUNKS)) as valD,
            tc.tile_pool(name="ps", bufs=2, space="PSUM") as psp,
        ):
            bias = const.tile([_P, 1], mybir.dt.float32)
            nc.gpsimd.memset(bias[:], -_S)
            ones_i8 = const.tile([_P, 2, _P], mybir.dt.int8)
            nc.gpsimd.memset(ones_i8[:], 60)  # 0x3C = e5m2 1.0
            onesT = ones_i8[:].bitcast(mybir.dt.float8e5)

            # fire ACT_TABLE_LOAD immediately (overlaps first DMA)
            warm = const.tile([_P, 1], mybir.dt.float32)
            nc.scalar.activation(out=warm[:], in_=bias[:],
                                 func=mybir.ActivationFunctionType.Exp,
                                 scale=1.0)

            psumA = psp.tile([_P, _B], mybir.dt.float32)
            psumD = psp.tile([_P, _B], mybir.dt.float32)

            n_a = sum(_A_CHUNKS)
            n_d = sum(_D_CHUNKS)
            ia = id_ = 0
            for eng, ci in _EMIT:
                if eng == "A":
                    npair, coff = _A_CHUNKS[ci], a_off[ci]
                else:
                    npair, coff = _D_CHUNKS[ci], d_off[ci]
                w = npair * 2 * _B
                pool = inpA if eng == "A" else inpD
                t = pool.tile([_P, w], mybir.dt.int8, tag=f"in{eng}")
                # A chunks issue on the GpSimd SWDGE queue, D on Sync: two
                # descriptor streams -> both engines stream concurrently.
                dma_eng = nc.gpsimd if eng == "A" else nc.sync
                dma_eng.dma_start(
                    out=t[:],
                    in_=x[coff : coff + _P * w].rearrange("(p w) -> p w", p=_P),
                )
                if eng == "A":
                    v = valA.tile([_P, w], mybir.dt.float8e5, tag="vA")
                    nc.scalar.activation(out=v[:], in_=t[:],
                                         func=mybir.ActivationFunctionType.Exp,
                                         scale=_S / _Q, bias=bias[:])
                else:
                    vb = valD.tile([_P, w], mybir.dt.int8, tag="vD")
                    nc.vector.tensor_scalar(out=vb[:], in0=t[:],
                                            scalar1=_SCH_A, scalar2=_SCH_B,
                                            op0=mybir.AluOpType.mult,
                                            op1=mybir.AluOpType.add)
                    v = vb[:].bitcast(mybir.dt.float8e5)
                for j in range(npair):
                    rhs = (v[:, j * 2 * _B : (j + 1) * 2 * _B]
                           .rearrange("p (k n) -> p k n", k=2))
                    if eng == "A":
                        ps, first, last = psumA, ia == 0, ia == n_a - 1
                        ia += 1
                    else:
                        ps, first, last = psumD, id_ == 0, id_ == n_d - 1
                        id_ += 1
                    nc.tensor.matmul(ps[:], lhsT=onesT, rhs=rhs,
                                     start=first, stop=last,
                                     perf_mode=mybir.MatmulPerfMode.DoubleRow,
                                     skip_group_check=True)

            sb = const.tile([1, 2 * _B], mybir.dt.float32)
            nc.scalar.copy(out=sb[:, :_B], in_=psumA[0:1, :])
            nc.vector.tensor_copy(out=sb[:, _B:], in_=psumD[0:1, :])
            nc.sync.dma_start(out=out[:].rearrange("(a b) -> a b", a=1),
                              in_=sb[:])

    nc.compile()
    return nc


def _get_nc():
    if "nc" not in _nc_cache:
        _nc_cache["nc"] = _build_nc()
    return _nc_cache["nc"]


def _pack_core(x8_core_T):
    """[CPAD, B] int8 (class-major) -> flat blob: D chunks (pairs 0..29,
    clamped) then A chunks (pairs 30..48), each chunk a raveled [128, n*2*B]
    block (the device reads each chunk contiguously)."""
    pairs = x8_core_T.reshape(_NPAIRS, 2, _P, _B)
    parts = []
    p = 0
    for npair in _D_CHUNKS:
        blk = np.maximum(pairs[p : p + npair], _DCLAMP)  # [n, 2, 128, B]
        parts.append(np.ascontiguousarray(blk.transpose(2, 0, 1, 3)).ravel())
        p += npair
    for npair in _A_CHUNKS:
        blk = pairs[p : p + npair]
        parts.append(np.ascontiguousarray(blk.transpose(2, 0, 1, 3)).ravel())
        p += npair
    return np.concatenate(parts)


def _device_row_sums(logits, trace=False):
    """Returns (row_sums[B] float64 ~= sum_c exp(S*logits-S), results)."""
    from concourse.bass_utils import run_bass_kernel_spmd

    B, C = logits.shape
    assert B == _B and C == _NCORES * _CS
    x8 = np.rint(np.clip(logits, -1.0, 1.0) * _Q).astype(np.int8)

    nc = _get_nc()
    in_maps = []
    for c in range(_NCORES):
        xT = np.full((_CPAD, _B), -128, dtype=np.int8)
        xT[:_CS] = x8[:, c * _CS : (c + 1) * _CS].T
        in_maps.append({"x": _pack_core(xT)})
    r = run_bass_kernel_spmd(nc, in_maps, core_ids=list(range(_NCORES)),
                             trace=trace)
    total = np.zeros(_B, np.float64)
    for res in r.results:
        arr = res["sums"].astype(np.float64)
        total += _KA * arr[:_B] + _KD * arr[_B:]
    return total, r


def _sim_label_term(x8v, eng):
    """Device's contribution for one element (pre-K scaling), simulated."""
    import ml_dtypes

    if eng == "A":
        v = np.exp(np.float32(_S / _Q) * np.float32(x8v) - np.float32(_S))
        return float(np.float32(v).astype(ml_dtypes.float8_e5m2)
                     .astype(np.float64)) * _KA
    byte = np.rint(np.float32(_SCH_A) * np.float32(max(x8v, _DCLAMP))
                   + np.float32(_SCH_B)).astype(np.int8)
    return float(byte.view(ml_dtypes.float8_e5m2).astype(np.float64)) * _KD


def kernel(logits, labels):
    logits = np.ascontiguousarray(np.asarray(logits, dtype=np.float32))
    labels_i = np.asarray(labels).astype(np.int64)
    B, C = logits.shape

    total, _ = _device_row_sums(logits)

    rows = np.arange(B)
    t = logits[rows, labels_i].astype(np.float64)
    x8t = np.rint(np.clip(t, -1.0, 1.0) * _Q)
    thresh = float(np.cos(np.pi - _M2))
    ang = np.arccos(np.clip(t, -1.0 + _EPS, 1.0 - _EPS))
    cos_m = np.cos(ang + _M2)
    theta = np.where(t > thresh, cos_m, -2.0 - cos_m)

    # subtract what the device actually added for the label column
    sub = np.empty(B)
    for r_ in range(B):
        c = int(labels_i[r_])
        pair = (c % _CS) // 256
        sub[r_] = _sim_label_term(x8t[r_], _PAIR_ENG[pair])
    corrected = total - sub + np.exp(_S * theta - _S)
    loss_rows = _S + np.log(corrected) - _S * theta
    return np.array(loss_rows.mean(), dtype=np.float32)


# revision 10
# speedup vs baseline: 1.1012x; 1.1012x over previous
"""CombinedMarginLoss (ArcFace, m1=1, m2=0.5, m3=0, easy_margin) on 8 trn2 cores.

Math: loss = mean_b [ logsumexp_c(margin_logits[b,c]) - S*theta_b ] where
margin_logits[b,c] = S*logits[b,c] except the label column which is S*theta_b.
Logits are cosines in [-1, 1], so exp(S*x - S) in [e^-128, 1] needs no max pass.

The class dim is sharded across the 8 cores (partial-FC style); each core
computes per-row partial sums of exp(S*x - S) and the host does the O(B)
label/margin/log epilogue.

Device design (per core), chosen from trace analysis of the v0 kernel
(ScalarE+DVE saturated by exp+reduce work, TensorE idle, 8.8us sem-drain tail):

- Transposed layout: classes on SBUF partitions, batch rows on the free dim.
  Host ships int8-quantized logits (x8 = rint(127*x)) as a packed blob so
  every DMA is one fully contiguous [128, W] read.
- exp is computed elementwise only (no reduction work on ScalarE/DVE):
    A path (ScalarE): act Exp(S/Q * x8 - S) -> fp8 e5m2 values (sub-subnormal
      flushes to +0, which implements the tail clamp for free).
    D path (VectorE): Schraudolph bit trick in fp8: byte = rint(A'*x8 + B')
      bitcast e5m2 ~= 2^15 * exp(S*x8/Q - S).  tensor_scalar int8->int8 runs
      in the DVE 2x_2P mode (2 elem/cycle/lane).  Host pre-clamps D-chunk
      inputs at x8=86 so the byte is always in [0, 120] (no NaN patterns).
- All reductions run on the otherwise-idle TensorE: fp8 DoubleRow matmul
  against an all-ones [128,2,128] stationary contracts 256 classes/pass at
  2 fp8/cell/cycle into PSUM (fp32), accumulated across all pairs.
  Separate PSUM banks for the A and D paths (different calibration).
- Host epilogue: sums = KA*A + KD*D per row per core; KA/KD are
  HW-calibrated constants absorbing fp8 rounding/Schraudolph/act-spline bias.
"""

import numpy as np

_S = 64.0
_M2 = 0.5
_EPS = 1e-7
_NCORES = 8
_P = 128
_B = 512  # batch rows
_CS = 12500  # classes per core
_NPAIRS = 49  # ceil(12500 / 256); last pair has 44 pad classes
_CPAD = _NPAIRS * 256  # 12544

_Q = 127.0
# D path (DVE bit trick): byte = rint(SCH_A * x8 + SCH_B) bitcast e5m2.
# byte(127) = 120 (= 2^15), byte(86) = 1; host clamps D inputs at 86.
_SCH_A = float(np.float32(4 * _S * np.log2(np.e) / _Q))  # 2.9081097
_SCH_B = -249.32993
_DCLAMP = 86

# Calibration constants: sums_true ~= KA * A_dev + KD * D_dev.
# Seeded from numpy simulation; refined on HW (see calib in transcript).
_KA = 0.9903318
_KD = 0.9855134 / 32768.0

# Per-core chunk plan: (engine, n_pairs) in DMA order, all on the Sync DMA
# queue (gpsimd SWDGE measured ~2x slower descriptor generation).  Finely
# interleaved so each engine's data arrives at the rate it can consume
# (DVE is ~1.6x faster per element than ScalarE -> 30/19 pair split), with
# 1-pair chunks at the head (fast pipeline fill) and tail (short drain).
_PLAN = [
    ("D", 1), ("A", 1), ("D", 2), ("A", 2), ("D", 3), ("A", 3), ("D", 4),
    ("A", 3), ("D", 4), ("A", 3), ("D", 4), ("A", 3), ("D", 4), ("A", 2),
    ("D", 4), ("A", 1), ("D", 3), ("A", 1), ("D", 1),
]
assert sum(n for e, n in _PLAN if e == "D") == 30
assert sum(n for e, n in _PLAN if e == "A") == 19

_PAIR_ENG = [e for e, n in _PLAN for _ in range(n)]

_nc_cache = {}


def _build_nc():
    import concourse.bacc as bacc
    import concourse.mybir as mybir
    from concourse.tile import TileContext

    nc = bacc.Bacc("TRN2", target_bir_lowering=False)
    x = nc.dram_tensor("x", [_P * _NPAIRS * 2 * _B], mybir.dt.int8,
                       kind="ExternalInput")
    out = nc.dram_tensor("sums", [2 * _B], mybir.dt.float32,
                         kind="ExternalOutput")

    n_ca = sum(1 for e, _ in _PLAN if e == "A")
    n_cd = sum(1 for e, _ in _PLAN if e == "D")
    with TileContext(nc) as tc:
        with (
            tc.tile_pool(name="const", bufs=1) as const,
            tc.tile_pool(name="inA", bufs=n_ca) as inpA,
            tc.tile_pool(name="inD", bufs=n_cd) as inpD,
            tc.tile_pool(name="vA", bufs=n_ca) as valA,
            tc.tile_pool(name="vD", bufs=n_cd) as valD,
            tc.tile_pool(name="ps", bufs=2, space="PSUM") as psp,
        ):
            bias = const.tile([_P, 1], mybir.dt.float32)
            nc.gpsimd.memset(bias[:], -_S)
            ones_i8 = const.tile([_P, 2, _P], mybir.dt.int8)
            nc.gpsimd.memset(ones_i8[:], 60)  # 0x3C = e5m2 1.0
            onesT = ones_i8[:].bitcast(mybir.dt.float8e5)

            # fire ACT_TABLE_LOAD immediately (overlaps first DMA)
            warm = const.tile([_P, 1], mybir.dt.float32)
            nc.scalar.activation(out=warm[:], in_=bias[:],
                                 func=mybir.ActivationFunctionType.Exp,
                                 scale=1.0)

            psumA = psp.tile([_P, _B], mybir.dt.float32)
            psumD = psp.tile([_P, _B], mybir.dt.float32)

            n_a = sum(1 for e in _PAIR_ENG if e == "A")
            n_d = sum(1 for e in _PAIR_ENG if e == "D")
            ia = id_ = 0
            off = 0
            for eng, npair in _PLAN:
                w = npair * 2 * _B
                pool = inpA if eng == "A" else inpD
                t = pool.tile([_P, w], mybir.dt.int8, tag=f"in{eng}")
                nc.sync.dma_start(
                    out=t[:],
                    in_=x[off : off + _P * w].rearrange("(p w) -> p w", p=_P),
                )
                off += _P * w
                if eng == "A":
                    v = valA.tile([_P, w], mybir.dt.float8e5, tag="vA")
                    nc.scalar.activation(out=v[:], in_=t[:],
                                         func=mybir.ActivationFunctionType.Exp,
                                         scale=_S / _Q, bias=bias[:])
                else:
                    vb = valD.tile([_P, w], mybir.dt.int8, tag="vD")
                    nc.vector.tensor_scalar(out=vb[:], in0=t[:],
                                            scalar1=_SCH_A, scalar2=_SCH_B,
                                            op0=mybir.AluOpType.mult,
                                            op1=mybir.AluOpType.add)
                    v = vb[:].bitcast(mybir.dt.float8e5)
                for j in range(npair):
                    rhs = (v[:, j * 2 * _B : (j + 1) * 2 * _B]
                           .rearrange("p (k n) -> p k n", k=2))
                    if eng == "A":
                        ps, first, last = psumA, ia == 0, ia == n_a - 1
                        ia += 1
                    else:
                        ps, first, last = psumD, id_ == 0, id_ == n_d - 1
                        id_ += 1
                    nc.tensor.matmul(ps[:], lhsT=onesT, rhs=rhs,
                                     start=first, stop=last,
                                     perf_mode=mybir.MatmulPerfMode.DoubleRow,
                                     skip_group_check=True)

            sb = const.tile([1, 2 * _B], mybir.dt.float32)
            nc.scalar.copy(out=sb[:, :_B], in_=psumA[0:1, :])
            nc.vector.tensor_copy(out=sb[:, _B:], in_=psumD[0:1, :])
            nc.sync.dma_start(out=out[:].rearrange("(a b) -> a b", a=1),
                              in_=sb[:])

    nc.compile()
    return nc


def _get_nc():
    if "nc" not in _nc_cache:
        _nc_cache["nc"] = _build_nc()
    return _nc_cache["nc"]


def _pack_core(x8_core_T):
    """[CPAD, B] int8 (class-major) -> flat blob in chunk DMA order; each
    chunk is a raveled [128, n*2*B] block (device reads it contiguously)."""
    pairs = x8_core_T.reshape(_NPAIRS, 2, _P, _B)
    parts = []
    p = 0
    for eng, npair in _PLAN:
        blk = pairs[p : p + npair]  # [n, 2, 128, B]
        if eng == "D":
            blk = np.maximum(blk, _DCLAMP)
        parts.append(np.ascontiguousarray(blk.transpose(2, 0, 1, 3)).ravel())
        p += npair
    return np.concatenate(parts)


def _device_row_sums(logits, trace=False):
    """Returns (row_sums[B] float64 ~= sum_c exp(S*logits-S), results)."""
    from concourse.bass_utils import run_bass_kernel_spmd

    B, C = logits.shape
    assert B == _B and C == _NCORES * _CS
    x8 = np.rint(np.clip(logits, -1.0, 1.0) * _Q).astype(np.int8)

    nc = _get_nc()
    in_maps = []
    for c in range(_NCORES):
        xT = np.full((_CPAD, _B), -128, dtype=np.int8)
        xT[:_CS] = x8[:, c * _CS : (c + 1) * _CS].T
        in_maps.append({"x": _pack_core(xT)})
    r = run_bass_kernel_spmd(nc, in_maps, core_ids=list(range(_NCORES)),
                             trace=trace)
    total = np.zeros(_B, np.float64)
    for res in r.results:
        arr = res["sums"].astype(np.float64)
        total += _KA * arr[:_B] + _KD * arr[_B:]
    return total, r


def _sim_label_term(x8v, eng):
    """Device's contribution for one element (pre-K scaling), simulated."""
    import ml_dtypes

    if eng == "A":
        v = np.exp(np.float32(_S / _Q) * np.float32(x8v) - np.float32(_S))
        return float(np.float32(v).astype(ml_dtypes.float8_e5m2)
                     .astype(np.float64)) * _KA
    byte = np.rint(np.float32(_SCH_A) * np.float32(max(x8v, _DCLAMP))
                   + np.float32(_SCH_B)).astype(np.int8)
    return float(byte.view(ml_dtypes.float8_e5m2).astype(np.float64)) * _KD


def kernel(logits, labels):
    logits = np.ascontiguousarray(np.asarray(logits, dtype=np.float32))
    labels_i = np.asarray(labels).astype(np.int64)
    B, C = logits.shape

    total, _ = _device_row_sums(logits)

    rows = np.arange(B)
    t = logits[rows, labels_i].astype(np.float64)
    x8t = np.rint(np.clip(t, -1.0, 1.0) * _Q)
    thresh = float(np.cos(np.pi - _M2))
    ang = np.arccos(np.clip(t, -1.0 + _EPS, 1.0 - _EPS))
    cos_m = np.cos(ang + _M2)
    theta = np.where(t > thresh, cos_m, -2.0 - cos_m)

    # subtract what the device actually added for the label column
    sub = np.empty(B)
    for r_ in range(B):
        c = int(labels_i[r_])
        pair = (c % _CS) // 256
        sub[r_] = _sim_label_term(x8t[r_], _PAIR_ENG[pair])
    corrected = total - sub + np.exp(_S * theta - _S)
    loss_rows = _S + np.log(corrected) - _S * theta
    return np.array(loss_rows.mean(), dtype=np.float32)
